# revision 4
# baseline (speedup 1.0000x reference)
"""Trainium2 Bass kernel for nn_DifferentialNoise.

Op (per reference): flatten each [W,H] map row-major into pairs (a, b);
out_even = a, out_odd = b - a/50. Purely elementwise over independent
length-2 groups -> shard batch dim (128) across 8 cores, 16 batches each.

Per core: 16 MiB in + 16 MiB out, memory-bound. Strategy: contiguous
[128, G, 2] fp32 tiles, compute in place (evens untouched; odds get
`odd - even*(1/50)`), so there is no separate output copy.
"""

import numpy as np

import concourse.bacc as bacc
import concourse.mybir as mybir
from concourse.bass_utils import run_bass_kernel_spmd
from concourse.tile import TileContext

N_CORES = 8
B, C, W, H = 128, 64, 64, 64
B_LOCAL = B // N_CORES  # 16
PER_CORE_ELEMS = B_LOCAL * C * W * H  # 4,194,304 (16 MiB fp32)

P = 128  # SBUF partitions
F = 4096  # free elems per partition per tile (tile = 2 MiB)
G = F // 2  # pairs per partition per tile
NT = PER_CORE_ELEMS // (P * F)  # 8 tiles
INV_N = 1.0 / 50.0

_cache = {}


def build_nc(nt=NT, g=G, bufs=4):
    per_core = P * g * 2 * nt
    nc = bacc.Bacc(
        "TRN2",
        target_bir_lowering=False,
        debug=False,
        enable_asserts=False,
        num_devices=N_CORES,
    )
    x = nc.dram_tensor("x", [per_core], mybir.dt.float32, kind="ExternalInput").ap()
    out = nc.dram_tensor(
        "out", [per_core], mybir.dt.float32, kind="ExternalOutput"
    ).ap()
    xv = x.rearrange("(n p g t) -> n p g t", n=nt, p=P, g=g, t=2)
    ov = out.rearrange("(n p g t) -> n p g t", n=nt, p=P, g=g, t=2)

    with TileContext(nc) as tc:
        with (
            tc.tile_pool(name="data", bufs=bufs) as pool,
            tc.tile_pool(name="tmp", bufs=bufs) as tpool,
        ):
            for n in range(nt):
                t = pool.tile([P, g, 2], mybir.dt.float32)
                nc.sync.dma_start(t[:], xv[n])
                tmp = tpool.tile([P, g], mybir.dt.float32)
                # tmp = even * (1/50) on ScalarE; odd -= tmp on VectorE.
                nc.scalar.mul(tmp[:], t[:, :, 0], INV_N)
                nc.vector.tensor_tensor(
                    t[:, :, 1], t[:, :, 1], tmp[:], mybir.AluOpType.subtract
                )
                nc.sync.dma_start(ov[n], t[:])
    nc.compile()
    return nc


def _run(x, trace=False, **kw):
    if "nc" not in _cache:
        _cache["nc"] = build_nc()
    nc = _cache["nc"]
    xs = np.ascontiguousarray(np.asarray(x, dtype=np.float32)).reshape(
        N_CORES, PER_CORE_ELEMS
    )
    in_maps = [{"x": xs[i]} for i in range(N_CORES)]
    res = run_bass_kernel_spmd(nc, in_maps, list(range(N_CORES)), trace=trace, **kw)
    out = np.concatenate([r["out"] for r in res.results]).reshape(B, C, W, H)
    return out, res


def kernel(x):
    out, _ = _run(x, trace=False)
    return out


# revision 5
# speedup vs baseline: 1.1210x; 1.1210x over previous
"""Trainium2 Bass kernel for nn_DifferentialNoise.

Op (per reference): flatten each [W,H] map row-major into pairs (a, b);
out_even = a, out_odd = b - a/50. Purely elementwise over independent
length-2 groups -> shard batch dim (128) across 8 cores, 16 batches each.

Per core: 16 MiB in + 16 MiB out, memory-bound. Strategy: contiguous
[128, G, 2] fp32 tiles, compute in place (evens untouched; odds get
`odd - even*(1/50)`), so there is no separate output copy.
"""

import numpy as np

import concourse.bacc as bacc
import concourse.mybir as mybir
from concourse.bass_utils import run_bass_kernel_spmd
from concourse.tile import TileContext

N_CORES = 8
B, C, W, H = 128, 64, 64, 64
B_LOCAL = B // N_CORES  # 16
PER_CORE_ELEMS = B_LOCAL * C * W * H  # 4,194,304 (16 MiB fp32)

P = 128  # SBUF partitions
F = 4096  # free elems per partition per tile (tile = 2 MiB)
G = F // 2  # pairs per partition per tile
NT = PER_CORE_ELEMS // (P * F)  # 8 tiles
INV_N = 1.0 / 50.0

_cache = {}


def build_nc(per_core=PER_CORE_ELEMS, f=F, bufs=6, split_last=2, mul_engine="vector"):
    """Tiles of [P, f] flat elems; the last tile is split into `split_last`
    pieces to shorten the pipeline drain. Loads on SP's HWDGE ring, stores
    on ACT's ring, compute on DVE (or ACT for the mul if mul_engine=scalar).
    """
    nc = bacc.Bacc(
        "TRN2",
        target_bir_lowering=False,
        debug=False,
        enable_asserts=False,
        num_devices=N_CORES,
    )
    x = nc.dram_tensor("x", [per_core], mybir.dt.float32, kind="ExternalInput").ap()
    out = nc.dram_tensor(
        "out", [per_core], mybir.dt.float32, kind="ExternalOutput"
    ).ap()

    nt = per_core // (P * f)
    tiles = [(n * P * f, f) for n in range(nt)]
    if split_last > 1:
        off, tf = tiles.pop()
        sf = tf // split_last
        for s in range(split_last):
            tiles.append((off + s * P * sf, sf))

    with TileContext(nc) as tc:
        with (
            tc.tile_pool(name="data", bufs=bufs) as pool,
            tc.tile_pool(name="tmp", bufs=bufs) as tpool,
        ):
            for off, tf in tiles:
                g = tf // 2
                xv = x[off : off + P * tf].rearrange("(p g t) -> p g t", p=P, g=g, t=2)
                ov = out[off : off + P * tf].rearrange(
                    "(p g t) -> p g t", p=P, g=g, t=2
                )
                t = pool.tile([P, g, 2], mybir.dt.float32, tag="data")
                nc.sync.dma_start(t[:], xv)
                tmp = tpool.tile([P, g], mybir.dt.float32, tag="tmp")
                if mul_engine == "scalar":
                    nc.scalar.mul(tmp[:], t[:, :, 0], INV_N)
                else:
                    nc.vector.tensor_scalar(
                        tmp[:], t[:, :, 0], INV_N, None, mybir.AluOpType.mult
                    )
                nc.vector.tensor_tensor(
                    t[:, :, 1], t[:, :, 1], tmp[:], mybir.AluOpType.subtract
                )
                nc.scalar.dma_start(ov, t[:])
    nc.compile()
    return nc


def _run(x, trace=False, **kw):
    if "nc" not in _cache:
        _cache["nc"] = build_nc()
    nc = _cache["nc"]
    xs = np.ascontiguousarray(np.asarray(x, dtype=np.float32)).reshape(
        N_CORES, PER_CORE_ELEMS
    )
    in_maps = [{"x": xs[i]} for i in range(N_CORES)]
    res = run_bass_kernel_spmd(nc, in_maps, list(range(N_CORES)), trace=trace, **kw)
    out = np.concatenate([r["out"] for r in res.results]).reshape(B, C, W, H)
    return out, res


def kernel(x):
    out, _ = _run(x, trace=False)
    return out
